# revision 31
# baseline (speedup 1.0000x reference)
"""CAM (channel self-attention) kernel for Trainium2 — 8 NeuronCores, batch-parallel.

Math per batch element b (A = x[b] reshaped [N=4096, C=512]):
    G = A^T A                  [C, C]   (symmetric!)
    P = softmax_rows(G)        [C, C]
    Y = A P                    [N, C]
    out = gamma * Y + x

Sharding: data-parallel over batch — core i handles batch element i.
No cross-core communication needed.

Schedule (v9): the kernel is HBM-bound on both the 8 MiB input read and
the 8 MiB output write (358 GB/s/core), with a hard dependency between
them (P needs all of A).  The host stages x per core in partition-major
layout [128, NT, C] (and reads the output back the same way), so every
DMA descriptor moves 2-16 KiB contiguous per partition — large-
descriptor DMA sustains ~425 GB/s vs ~310 for the 2 KiB row-major
pattern.  fp8 is used ONLY where the PE consumes it (DoubleRow Y
matmuls, ~1.5x bf16 at FD=512) and ONLY produced by ACT — DVE/GpSimd
fp8 stores run ~6x slow, concurrent GpSimd tensor work contends DVE's
SBUF ports, and GpSimd SWDGE DMAs slow PE weight loads (all measured
on HW).
  - Input phase (Sync HWDGE ring): per 128-row chunk k, DVE cast ->
    A16 (bf16); 4 PE bf16 transposes -> PSUM -> one strided ACT copy
    into AT8 (ci-major; casts to fp8e4); upper-triangle bf16 Gram
    matmuls (free dims 512/384/256/128 exploiting G's symmetry) into 3
    PSUM banks.  8 warm-up matmuls (into the g0 bank, reset later by
    start=True) spin the PE HAM clock gate up (1.2 -> 2.4 GHz) during
    the DMA preamble window.
  - Lower triangle of G via 6 PE f32 transposes of the upper blocks.
  - softmax: DVE row-max (negated) -> ACT exp (bf16 E) with fused
    row-sum -> DVE reciprocal -> ACT scale-copy -> P8 (fp8).
  - Output phase (ACT HWDGE ring): Y = A P via 2 fp8 DoubleRow matmuls
    per row chunk (lhsT = AT8, rhs = P8, contraction 256/instr); DVE
    scalar_tensor_tensor epilogue out = (Y * gamma) + A32 in f32
    (exact x-residual), staged 512 KiB groups.
fp8 quantization only touches the gamma*Y term; tolerance is rel 2e-2.
"""

import numpy as np

import concourse.tile as tile
from concourse import bacc, mybir
from concourse.bass_utils import run_bass_kernel_spmd
from concourse.masks import make_identity

B = 8
H = 64
W = 64
C = 512
HW = H * W            # 4096 rows per batch element
NT = HW // 128        # 32 row chunks of 128
CT = C // 128         # 4 col chunks of 128
GRP = 4               # row chunks per input DMA group (1 MiB)
OGRP = 2              # row chunks per output DMA group (512 KiB)

F32 = mybir.dt.float32
BF16 = mybir.dt.bfloat16
FP8 = mybir.dt.float8e4
DR = mybir.MatmulPerfMode.DoubleRow

_CACHE = {}


def _emit(nc, tc, out, x, gamma):
    from contextlib import ExitStack

    with ExitStack() as ctx:
        big = ctx.enter_context(tc.tile_pool(name="big", bufs=1))
        small = ctx.enter_context(tc.tile_pool(name="small", bufs=1))
        stat = ctx.enter_context(tc.tile_pool(name="stat", bufs=4))
        ostage = ctx.enter_context(tc.tile_pool(name="ostage", bufs=4))
        gps = ctx.enter_context(tc.tile_pool(name="gps", bufs=1, space="PSUM"))
        wps = ctx.enter_context(tc.tile_pool(name="wps", bufs=5, space="PSUM"))

        A32 = big.tile([128, NT, C], F32)     # x rows, n on partitions (exact)
        A16 = big.tile([128, NT, C], BF16)    # bf16 cast of A32
        AT8 = big.tile([128, CT, HW], FP8)    # A^T, c on partitions
        G32 = big.tile([128, CT, C], F32)     # full Gram matrix in SBUF
        E16 = big.tile([128, CT, C], BF16)    # exp(G - rowmax)
        P8 = big.tile([128, CT, C], FP8)      # softmax(G) in fp8

        # Upper-triangle Gram accumulators: G[mi-chunk, mi*128:].
        # g1 (384 cols) and g3 (128 cols) share one PSUM bank.
        g0 = gps.tile([128, C], F32, name="g0", tag="g0")
        g13 = gps.tile([128, C], F32, name="g13", tag="g13")
        g2 = gps.tile([128, C - 256], F32, name="g2", tag="g2")
        g_ps = [g0[:], g13[:, 0:384], g2[:], g13[:, 384:512]]

        # PE warm-up first (needs no identity): the HAM clock gate holds the
        # PE at 1.2 GHz until it has been busy ~3.4us; burn that during the
        # DMA preamble window.  Warm matmuls write into the g0 bank, which
        # the first real Gram matmul resets via start=True.  The source is
        # memset on DVE in bf16 (fast store path, empty early queue) so the
        # warm-up isn't gated behind GpSimd's identity-building ops.
        warm_src = small.tile([128, C], BF16)
        nc.vector.memset(warm_src[:], 0.0)
        for wi in range(8):
            nc.tensor.matmul(
                g0[:], warm_src[:, 0:128], warm_src[:],
                start=(wi == 0), stop=(wi == 7),
            )

        ident = small.tile([128, 128], BF16)
        make_identity(nc, ident[:])
        ident32 = small.tile([128, 128], F32)
        make_identity(nc, ident32[:])

        gB = small.tile([128, 1], F32)        # gamma broadcast to all partitions

        # First loads chunk-granular so the PE can start early, then 1 MiB.
        load_groups = [1, 1, 2] + [GRP] * ((NT - 4) // GRP)
        assert sum(load_groups) == NT
        k0 = 0
        for gi, gsz in enumerate(load_groups):
            nc.sync.dma_start(A32[:, k0:k0 + gsz, :], x[:, k0:k0 + gsz, :])
            if gi == 0:
                # gamma: tiny load on the ACT HWDGE ring, off the input path
                nc.scalar.dma_start(gB[:], gamma[:])
            for j in range(gsz):
                k = k0 + j
                # cast f32 -> bf16 (DVE)
                nc.vector.tensor_copy(A16[:, k, :], A32[:, k, :])
                # A^T blocks of this chunk -> one PSUM bank, one strided copy
                # on ACT (the only engine with fast fp8 stores)
                tp = wps.tile([128, CT * 128], BF16, name="tp", tag="w")
                for ci in range(CT):
                    nc.tensor.transpose(
                        tp[:, ci * 128:(ci + 1) * 128],
                        A16[:, k, ci * 128:(ci + 1) * 128],
                        ident[:],
                    )
                nc.scalar.copy(
                    AT8[:, :, k * 128:(k + 1) * 128],
                    tp[:].rearrange("p (ci n) -> p ci n", ci=CT),
                )
                # upper-triangle Gram matmuls for this chunk (bf16)
                for mi in range(CT):
                    nc.tensor.matmul(
                        g_ps[mi],
                        A16[:, k, mi * 128:(mi + 1) * 128],
                        A16[:, k, mi * 128:],
                        start=(k == 0),
                        stop=(k == NT - 1),
                        # g1/g3 share a bank; per-element has_written makes
                        # disjoint-region groups safe on HW
                        skip_group_check=(mi % 2 == 1),
                    )
            k0 += gsz

        # G (upper) PSUM -> SBUF
        for mi in range(CT):
            if mi % 2 == 0:
                nc.vector.tensor_copy(G32[:, mi, mi * 128:], g_ps[mi])
            else:
                nc.scalar.copy(G32[:, mi, mi * 128:], g_ps[mi])
        # reconstruct lower triangle: G[mi, j] = G[j, mi]^T for j < mi
        for mi in range(1, CT):
            for j in range(mi):
                lb = wps.tile([128, 128], F32, name="lb", tag="w")
                nc.tensor.transpose(
                    lb[:], G32[:, j, mi * 128:(mi + 1) * 128], ident32[:])
                if (mi + j) % 2 == 0:
                    nc.vector.tensor_copy(G32[:, mi, j * 128:(j + 1) * 128], lb[:])
                else:
                    nc.scalar.copy(G32[:, mi, j * 128:(j + 1) * 128], lb[:])

        # softmax over rows of G (free axis); P8 written by ACT scale-copy
        # (the only engine with fast fp8 stores)
        for mi in range(CT):
            nmax = stat.tile([128, 1], F32)
            nc.vector.tensor_reduce(
                nmax[:], G32[:, mi, :],
                axis=mybir.AxisListType.X, op=mybir.AluOpType.max, negate=True,
            )
            esum = stat.tile([128, 1], F32)
            nc.scalar.activation(
                E16[:, mi, :], G32[:, mi, :],
                mybir.ActivationFunctionType.Exp,
                bias=nmax[:], scale=1.0, accum_out=esum[:],
            )
            rsum = stat.tile([128, 1], F32)
            nc.vector.reciprocal(rsum[:], esum[:])
            nc.scalar.mul(P8[:, mi, :], E16[:, mi, :], rsum[:])

        # Y = A @ P (fp8 DoubleRow), epilogue out = gamma * Y + x (f32, exact)
        out_groups = [OGRP] * (NT // OGRP - 1) + [1, 1]
        t0 = 0
        for h, osz in enumerate(out_groups):
            o32 = ostage.tile([128, OGRP, C], F32)
            for j in range(osz):
                t = t0 + j
                y = wps.tile([128, C], F32, name="y", tag="w")
                for cp in range(CT // 2):
                    nc.tensor.matmul(
                        y[:],
                        AT8[:, 2 * cp:2 * cp + 2, t * 128:(t + 1) * 128],
                        P8[:, 2 * cp:2 * cp + 2, :],
                        start=(cp == 0),
                        stop=(cp == CT // 2 - 1),
                        perf_mode=DR,
                    )
                nc.vector.scalar_tensor_tensor(
                    o32[:, j, :], y[:], gB[:], A32[:, t, :],
                    op0=mybir.AluOpType.mult, op1=mybir.AluOpType.add,
                )
            # all output on the ACT HWDGE ring (idle in phase 2; alternating
            # rings measured 242 GB/s vs 341 single-ring)
            nc.scalar.dma_start(out[:, t0:t0 + osz, :], o32[:, 0:osz, :])
            t0 += osz


def build():
    nc = bacc.Bacc("TRN2", target_bir_lowering=False, debug=False)
    # partition-major DRAM layout: [p, t, c]; the host pre/post-shuffles
    x = nc.dram_tensor("x", [128, NT, C], F32, kind="ExternalInput").ap()
    gamma = nc.dram_tensor("gamma", [128, 1], F32, kind="ExternalInput").ap()
    out = nc.dram_tensor("out", [128, NT, C], F32, kind="ExternalOutput").ap()
    with tile.TileContext(nc) as tc:
        _emit(nc, tc, out, x, gamma)
    nc.compile()
    return nc


def kernel(x: np.ndarray, gamma: np.ndarray, trace: bool = False):
    assert x.shape == (B, H, W, C), x.shape
    if "nc" not in _CACHE:
        _CACHE["nc"] = build()
    nc = _CACHE["nc"]

    g128 = np.full((128, 1), np.float32(np.asarray(gamma).reshape(-1)[0]),
                   dtype=np.float32)
    in_maps = [
        {
            # [HW, C] -> partition-major [128, NT, C]: row r = t*128 + p
            "x": np.ascontiguousarray(
                np.asarray(x[i], dtype=np.float32)
                .reshape(NT, 128, C).transpose(1, 0, 2)),
            "gamma": g128,
        }
        for i in range(B)
    ]
    if trace:
        res = run_bass_kernel_spmd(nc, in_maps, core_ids=list(range(B)),
                                   trace=True)
    else:
        # Force-untraced: a stray BASS_TRACE in the environment would route
        # through profiling hooks this image may not have.
        import os
        prev = os.environ.get("BASS_NEVER_TRACE")
        os.environ["BASS_NEVER_TRACE"] = "1"
        try:
            res = run_bass_kernel_spmd(nc, in_maps, core_ids=list(range(B)))
        finally:
            if prev is None:
                os.environ.pop("BASS_NEVER_TRACE", None)
            else:
                os.environ["BASS_NEVER_TRACE"] = prev
    _CACHE["last_result"] = res
    # [128, NT, C] -> [HW, C]
    out = np.stack(
        [np.asarray(res.results[i]["out"]).reshape(128, NT, C)
         .transpose(1, 0, 2).reshape(HW, C) for i in range(B)],
        axis=0,
    )
    return out.reshape(B, H, W, C).astype(np.float32)


# revision 32
# speedup vs baseline: 1.0381x; 1.0381x over previous
"""CAM (channel self-attention) kernel for Trainium2 — 8 NeuronCores, batch-parallel.

Math per batch element b (A = x[b] reshaped [N=4096, C=512]):
    G = A^T A                  [C, C]   (symmetric!)
    P = softmax_rows(G)        [C, C]
    Y = A P                    [N, C]
    out = gamma * Y + x

Sharding: data-parallel over batch — core i handles batch element i.
No cross-core communication needed.

Schedule (v9): the kernel is HBM-bound on both the 8 MiB input read and
the 8 MiB output write (358 GB/s/core), with a hard dependency between
them (P needs all of A).  The host stages x per core in partition-major
layout [128, NT, C] (and reads the output back the same way), so every
DMA descriptor moves 2-16 KiB contiguous per partition — large-
descriptor DMA sustains ~425 GB/s vs ~310 for the 2 KiB row-major
pattern.  fp8 is used ONLY where the PE consumes it (DoubleRow Y
matmuls, ~1.5x bf16 at FD=512) and ONLY produced by ACT — DVE/GpSimd
fp8 stores run ~6x slow, concurrent GpSimd tensor work contends DVE's
SBUF ports, and GpSimd SWDGE DMAs slow PE weight loads (all measured
on HW).
  - Input phase (Sync HWDGE ring): per 128-row chunk k, DVE cast ->
    A16 (bf16); 4 PE bf16 transposes -> PSUM -> one strided ACT copy
    into AT8 (ci-major; casts to fp8e4); upper-triangle bf16 Gram
    matmuls (free dims 512/384/256/128 exploiting G's symmetry) into 3
    PSUM banks.  8 warm-up matmuls (into the g0 bank, reset later by
    start=True) spin the PE HAM clock gate up (1.2 -> 2.4 GHz) during
    the DMA preamble window.
  - Lower triangle of G via 6 PE f32 transposes of the upper blocks.
  - softmax: DVE row-max (negated) -> ACT exp (bf16 E) with fused
    row-sum -> DVE reciprocal -> ACT scale-copy -> P8 (fp8).
  - Output phase (ACT HWDGE ring): Y = A P via 2 fp8 DoubleRow matmuls
    per row chunk (lhsT = AT8, rhs = P8, contraction 256/instr); DVE
    scalar_tensor_tensor epilogue out = (Y * gamma) + A32 in f32
    (exact x-residual), staged 512 KiB groups.
fp8 quantization only touches the gamma*Y term; tolerance is rel 2e-2.
"""

import numpy as np

import concourse.tile as tile
from concourse import bacc, mybir
from concourse.bass_utils import run_bass_kernel_spmd
from concourse.masks import make_identity

B = 8
H = 64
W = 64
C = 512
HW = H * W            # 4096 rows per batch element
NT = HW // 128        # 32 row chunks of 128
CT = C // 128         # 4 col chunks of 128
GRP = 4               # row chunks per input DMA group (1 MiB)
OGRP = 2              # row chunks per output DMA group (512 KiB)

F32 = mybir.dt.float32
BF16 = mybir.dt.bfloat16
FP8 = mybir.dt.float8e4
DR = mybir.MatmulPerfMode.DoubleRow

_CACHE = {}


def _emit(nc, tc, out, x, gamma):
    from contextlib import ExitStack

    with ExitStack() as ctx:
        big = ctx.enter_context(tc.tile_pool(name="big", bufs=1))
        small = ctx.enter_context(tc.tile_pool(name="small", bufs=1))
        stat = ctx.enter_context(tc.tile_pool(name="stat", bufs=4))
        ostage = ctx.enter_context(tc.tile_pool(name="ostage", bufs=4))
        gps = ctx.enter_context(tc.tile_pool(name="gps", bufs=1, space="PSUM"))
        wps = ctx.enter_context(tc.tile_pool(name="wps", bufs=5, space="PSUM"))

        A32 = big.tile([128, NT, C], F32)     # x rows, n on partitions (exact)
        A16 = big.tile([128, NT, C], BF16)    # bf16 cast of A32
        AT8 = big.tile([128, CT, HW], FP8)    # A^T, c on partitions
        G32 = big.tile([128, CT, C], F32)     # full Gram matrix in SBUF
        E16 = big.tile([128, CT, C], BF16)    # exp(G - rowmax)
        P8 = big.tile([128, CT, C], FP8)      # softmax(G) in fp8

        # Upper-triangle Gram accumulators: G[mi-chunk, mi*128:].
        # g1 (384 cols) and g3 (128 cols) share one PSUM bank.
        g0 = gps.tile([128, C], F32, name="g0", tag="g0")
        g13 = gps.tile([128, C], F32, name="g13", tag="g13")
        g2 = gps.tile([128, C - 256], F32, name="g2", tag="g2")
        g_ps = [g0[:], g13[:, 0:384], g2[:], g13[:, 384:512]]

        # PE warm-up first (needs no identity): the HAM clock gate holds the
        # PE at 1.2 GHz until it has been busy ~3.4us; burn that during the
        # DMA preamble window.  Warm matmuls write into the g0 bank, which
        # the first real Gram matmul resets via start=True.
        warm_src = small.tile([128, C], FP8)
        nc.gpsimd.memset(warm_src[:], 0.0)
        for wi in range(8):
            nc.tensor.matmul(
                g0[:], warm_src[:, 0:128], warm_src[:],
                start=(wi == 0), stop=(wi == 7),
            )

        ident = small.tile([128, 128], BF16)
        make_identity(nc, ident[:])
        ident32 = small.tile([128, 128], F32)
        make_identity(nc, ident32[:])

        gB = small.tile([128, 1], F32)        # gamma broadcast to all partitions

        # First loads chunk-granular so the PE can start early, then 1 MiB.
        load_groups = [1, 1, 2] + [GRP] * ((NT - 4) // GRP)
        assert sum(load_groups) == NT
        k0 = 0
        for gi, gsz in enumerate(load_groups):
            nc.sync.dma_start(A32[:, k0:k0 + gsz, :], x[:, k0:k0 + gsz, :])
            if gi == 0:
                # gamma: tiny load on the ACT HWDGE ring, off the input path
                nc.scalar.dma_start(gB[:], gamma[:])
            for j in range(gsz):
                k = k0 + j
                # cast f32 -> bf16 (DVE)
                nc.vector.tensor_copy(A16[:, k, :], A32[:, k, :])
                # A^T blocks of this chunk -> one PSUM bank, one strided copy
                # on ACT (the only engine with fast fp8 stores)
                tp = wps.tile([128, CT * 128], BF16, name="tp", tag="w")
                for ci in range(CT):
                    nc.tensor.transpose(
                        tp[:, ci * 128:(ci + 1) * 128],
                        A16[:, k, ci * 128:(ci + 1) * 128],
                        ident[:],
                    )
                nc.scalar.copy(
                    AT8[:, :, k * 128:(k + 1) * 128],
                    tp[:].rearrange("p (ci n) -> p ci n", ci=CT),
                )
                # upper-triangle Gram matmuls for this chunk (bf16)
                for mi in range(CT):
                    nc.tensor.matmul(
                        g_ps[mi],
                        A16[:, k, mi * 128:(mi + 1) * 128],
                        A16[:, k, mi * 128:],
                        start=(k == 0),
                        stop=(k == NT - 1),
                        # g1/g3 share a bank; per-element has_written makes
                        # disjoint-region groups safe on HW
                        skip_group_check=(mi % 2 == 1),
                    )
            k0 += gsz

        # G (upper) PSUM -> SBUF
        for mi in range(CT):
            if mi % 2 == 0:
                nc.vector.tensor_copy(G32[:, mi, mi * 128:], g_ps[mi])
            else:
                nc.scalar.copy(G32[:, mi, mi * 128:], g_ps[mi])
        # reconstruct lower triangle: G[mi, j] = G[j, mi]^T for j < mi
        for mi in range(1, CT):
            for j in range(mi):
                lb = wps.tile([128, 128], F32, name="lb", tag="w")
                nc.tensor.transpose(
                    lb[:], G32[:, j, mi * 128:(mi + 1) * 128], ident32[:])
                if (mi + j) % 2 == 0:
                    nc.vector.tensor_copy(G32[:, mi, j * 128:(j + 1) * 128], lb[:])
                else:
                    nc.scalar.copy(G32[:, mi, j * 128:(j + 1) * 128], lb[:])

        # softmax over rows of G (free axis); P8 written by ACT scale-copy
        # (the only engine with fast fp8 stores)
        for mi in range(CT):
            nmax = stat.tile([128, 1], F32)
            nc.vector.tensor_reduce(
                nmax[:], G32[:, mi, :],
                axis=mybir.AxisListType.X, op=mybir.AluOpType.max, negate=True,
            )
            esum = stat.tile([128, 1], F32)
            nc.scalar.activation(
                E16[:, mi, :], G32[:, mi, :],
                mybir.ActivationFunctionType.Exp,
                bias=nmax[:], scale=1.0, accum_out=esum[:],
            )
            rsum = stat.tile([128, 1], F32)
            nc.vector.reciprocal(rsum[:], esum[:])
            nc.scalar.mul(P8[:, mi, :], E16[:, mi, :], rsum[:])

        # Y = A @ P (fp8 DoubleRow), epilogue out = gamma * Y + x (f32, exact)
        out_groups = [OGRP] * (NT // OGRP - 1) + [1, 1]
        t0 = 0
        for h, osz in enumerate(out_groups):
            o32 = ostage.tile([128, OGRP, C], F32)
            for j in range(osz):
                t = t0 + j
                y = wps.tile([128, C], F32, name="y", tag="w")
                for cp in range(CT // 2):
                    nc.tensor.matmul(
                        y[:],
                        AT8[:, 2 * cp:2 * cp + 2, t * 128:(t + 1) * 128],
                        P8[:, 2 * cp:2 * cp + 2, :],
                        start=(cp == 0),
                        stop=(cp == CT // 2 - 1),
                        perf_mode=DR,
                    )
                nc.vector.scalar_tensor_tensor(
                    o32[:, j, :], y[:], gB[:], A32[:, t, :],
                    op0=mybir.AluOpType.mult, op1=mybir.AluOpType.add,
                )
            # all output on the ACT HWDGE ring (idle in phase 2; alternating
            # rings measured 242 GB/s vs 341 single-ring)
            nc.scalar.dma_start(out[:, t0:t0 + osz, :], o32[:, 0:osz, :])
            t0 += osz


def build():
    nc = bacc.Bacc("TRN2", target_bir_lowering=False, debug=False)
    # partition-major DRAM layout: [p, t, c]; the host pre/post-shuffles
    x = nc.dram_tensor("x", [128, NT, C], F32, kind="ExternalInput").ap()
    gamma = nc.dram_tensor("gamma", [128, 1], F32, kind="ExternalInput").ap()
    out = nc.dram_tensor("out", [128, NT, C], F32, kind="ExternalOutput").ap()
    with tile.TileContext(nc) as tc:
        _emit(nc, tc, out, x, gamma)
    nc.compile()
    return nc


def kernel(x: np.ndarray, gamma: np.ndarray, trace: bool = False):
    assert x.shape == (B, H, W, C), x.shape
    if "nc" not in _CACHE:
        _CACHE["nc"] = build()
    nc = _CACHE["nc"]

    g128 = np.full((128, 1), np.float32(np.asarray(gamma).reshape(-1)[0]),
                   dtype=np.float32)
    in_maps = [
        {
            # [HW, C] -> partition-major [128, NT, C]: row r = t*128 + p
            "x": np.ascontiguousarray(
                np.asarray(x[i], dtype=np.float32)
                .reshape(NT, 128, C).transpose(1, 0, 2)),
            "gamma": g128,
        }
        for i in range(B)
    ]
    if trace:
        res = run_bass_kernel_spmd(nc, in_maps, core_ids=list(range(B)),
                                   trace=True)
    else:
        # Force-untraced: a stray BASS_TRACE in the environment would route
        # through profiling hooks this image may not have.
        import os
        prev = os.environ.get("BASS_NEVER_TRACE")
        os.environ["BASS_NEVER_TRACE"] = "1"
        try:
            res = run_bass_kernel_spmd(nc, in_maps, core_ids=list(range(B)))
        finally:
            if prev is None:
                os.environ.pop("BASS_NEVER_TRACE", None)
            else:
                os.environ["BASS_NEVER_TRACE"] = prev
    _CACHE["last_result"] = res
    # [128, NT, C] -> [HW, C]
    out = np.stack(
        [np.asarray(res.results[i]["out"]).reshape(128, NT, C)
         .transpose(1, 0, 2).reshape(HW, C) for i in range(B)],
        axis=0,
    )
    return out.reshape(B, H, W, C).astype(np.float32)
